# revision 14
# baseline (speedup 1.0000x reference)
"""Trainium2 Bass kernel for nn_Attention (S=2048, B=2, D=1024, H=16, C=64).

Tensor-parallel over heads across 8 NeuronCores (2 heads/core), fully
pipelined wavefront:
  - p1 (projections+norm+rope, 8 chunks of 512 tokens): q/k/v = W.T @ x
    with Wq/Wk pre-scaled by the RMSNorm weights on host; sumsq via
    matmul against 1/w^2 selector columns; rstd = exp(-0.5*ln(ms/C+eps))
    so ScalarE only ever needs the {ln,exp} table set (no table swaps
    with the softmax exp); rstd broadcast to 128 partitions via a DRAM
    bounce with stride-0 reads; rope pairs swapped via SBUF-SBUF DMA.
  - p2 (attention, 8 query chunks of 512): scores computed transposed
    [keys, queries] per head with K=64 contraction (PE quadrant per
    head); one [128,1024] exp per key-block covering both heads;
    attn@v accumulated in PSUM with an appended ones column so the
    softmax denominator falls out; normalize = DVE reciprocal +
    gpsimd partition_broadcast + one multiply.
  - The AllToAll re-shard is split into 4 chunks (one per pair of query
    chunks), each fired as soon as its data is ready so the collective
    overlaps p2 compute; p3 (out projection vs full Wout) runs per
    received quarter. p1 chunks 4-7 and p3 quarters are interleaved
    into the p2 emission so every engine queue stays fed.
"""

import sys

if "/opt/trn_rl_repo" not in sys.path:
    sys.path.insert(0, "/opt/trn_rl_repo")

import numpy as np
import concourse.bass as bass
from concourse import bacc, tile, mybir
from concourse.bass_utils import run_bass_kernel_spmd
from concourse.masks import make_identity

S, B, D, H, C = 2048, 2, 1024, 16, 64
EPS = 1e-6
NCORES = 8
T = S * B                  # 4096 tokens, batch-major: t = b*S + s
LH = H // NCORES           # 2 local heads
LC = LH * C                # 128 local head columns
TCH = 512                  # p1/p2 token chunk
NCH = T // TCH             # 8
NJT = S // 128             # 16 key blocks per batch
TOK_OUT = T // NCORES      # 512 output tokens per core

F32 = mybir.dt.float32
F32R = mybir.dt.float32r
BF16 = mybir.dt.bfloat16
AF = mybir.ActivationFunctionType

_CACHE = {}
LAST_RESULTS = None


def _build():
    nc = bacc.Bacc("TRN2", target_bir_lowering=False, debug=False,
                   num_devices=NCORES)
    xT = nc.dram_tensor("xT", [D, T], F32, kind="ExternalInput")
    wq = nc.dram_tensor("wq", [D, LC], F32, kind="ExternalInput")
    wk = nc.dram_tensor("wk", [D, LC], F32, kind="ExternalInput")
    wv = nc.dram_tensor("wv", [D, LC], F32, kind="ExternalInput")
    wo = nc.dram_tensor("wo", [H * C, D], F32, kind="ExternalInput")
    ropeA = nc.dram_tensor("ropeA", [LC, T], F32, kind="ExternalInput")
    ropeB = nc.dram_tensor("ropeB", [LC, T], F32, kind="ExternalInput")
    winv = nc.dram_tensor("winv", [128, 8], F32, kind="ExternalInput")
    out = nc.dram_tensor("out", [TOK_OUT, D], F32, kind="ExternalOutput")

    xT4 = xT.rearrange("(a p) t -> p a t", p=128)       # [128, 8, T]
    wq4 = wq.rearrange("(a p) c -> p a c", p=128)       # [128, 8, LC]
    wk4 = wk.rearrange("(a p) c -> p a c", p=128)
    wv4 = wv.rearrange("(a p) c -> p a c", p=128)
    wo4 = wo.rearrange("(a p) n -> p a n", p=128)       # [128, 8, D]

    with tile.TileContext(nc) as tc:
        with (
            tc.tile_pool(name="singles", bufs=1) as singles,
            tc.tile_pool(name="xtp", bufs=2) as xtp,
            tc.tile_pool(name="ropep", bufs=2) as ropep,
            tc.tile_pool(name="workp", bufs=2) as workp,
            tc.tile_pool(name="ps1", bufs=2, space="PSUM") as ps1p,
            tc.tile_pool(name="pss", bufs=2, space="PSUM") as pssp,
            tc.tile_pool(name="pos", bufs=2, space="PSUM") as posp,
            tc.tile_pool(name="dram", bufs=1, space="DRAM") as dram,
        ):
            # ---- constants ----
            ident = singles.tile([128, 128], BF16)
            make_identity(nc, ident)
            winvf = singles.tile([128, 8], F32)
            nc.sync.dma_start(out=winvf, in_=winv[:, :])
            winv_sb = singles.tile([128, 8], F32R)
            nc.vector.tensor_copy(winv_sb, winvf)
            eps128 = singles.tile([128, 1], F32)
            nc.vector.memset(eps128, EPS)

            # ---- weights ----
            wq_sb = singles.tile([128, 8, LC], BF16)
            wk_sb = singles.tile([128, 8, LC], BF16)
            wv_sb = singles.tile([128, 8, LC], BF16)
            nc.gpsimd.dma_start(out=wq_sb, in_=wq4)
            nc.gpsimd.dma_start(out=wk_sb, in_=wk4)
            nc.gpsimd.dma_start(out=wv_sb, in_=wv4)
            wo_sb = singles.tile([128, 8, D], BF16)
            nc.gpsimd.dma_start(out=wo_sb, in_=wo4)

            # ---- persistent activations ----
            qT_sb = singles.tile([128, T], BF16)   # [2 heads x 64c, t]
            k_sb = singles.tile([128, T], BF16)    # [2 heads x 64c, t]
            # v with appended ones column per head: [t%128, t//128, lh, 65]
            v_sb = singles.tile([128, T // 128, LH, C + 1], BF16)
            onescol = singles.tile([128, T // 128, LH, 1], F32)
            nc.vector.memset(onescol, 1.0)
            nc.vector.tensor_copy(v_sb[:, :, :, C:C + 1], onescol)

            # DRAM bounce buffers for the 4 chunked AllToAlls
            bins = [dram.tile([NCORES * 128, 128], BF16, tag=f"bin{q}",
                              name=f"bin{q}") for q in range(4)]
            bouts = [dram.tile([NCORES * 128, 128], BF16, tag=f"bout{q}",
                               name=f"bout{q}") for q in range(4)]

            xts = {}
            ras = {}
            rbs = {}

            def load_xt(ch):
                xt = xtp.tile([128, 8, TCH], BF16, tag="xt",
                              name=f"xt{ch}")
                nc.gpsimd.dma_start(out=xt, in_=xT4[:, :, ch * TCH:
                                                    (ch + 1) * TCH])
                xts[ch] = xt

            def load_rope(ch):
                t0 = ch * TCH
                ra = ropep.tile([128, TCH], F32, tag="ra", name=f"ra{ch}")
                rb = ropep.tile([128, TCH], F32, tag="rb", name=f"rb{ch}")
                nc.gpsimd.dma_start(out=ra, in_=ropeA[:, t0:t0 + TCH])
                nc.gpsimd.dma_start(out=rb, in_=ropeB[:, t0:t0 + TCH])
                ras[ch] = ra
                rbs[ch] = rb

            # =============== phase-1 chunk (4 stages) =====================
            def p1_stages(ch):
                t0 = ch * TCH
                st = {}

                def stage_a():
                    if ch + 2 < NCH:
                        load_xt(ch + 2)
                    if ch + 1 < NCH:
                        load_rope(ch + 1)
                    psq = ps1p.tile([128, TCH], F32, tag="ps1",
                                    name=f"psq{ch}")
                    for a in range(8):
                        nc.tensor.matmul(psq, wq_sb[:, a, :],
                                         xts[ch][:, a, :],
                                         start=(a == 0), stop=(a == 7))
                    qraw = workp.tile([128, TCH], F32, tag="qraw",
                                      name=f"qraw{ch}")
                    nc.vector.tensor_copy(qraw, psq)
                    sq2 = workp.tile([128, TCH], F32R, tag="sq2",
                                     name=f"sq2_{ch}")
                    nc.gpsimd.tensor_mul(sq2, qraw, qraw)
                    st["qraw"], st["sq2"] = qraw, sq2

                def stage_b():
                    psk = ps1p.tile([128, TCH], F32, tag="ps1",
                                    name=f"psk{ch}")
                    for a in range(8):
                        nc.tensor.matmul(psk, wk_sb[:, a, :],
                                         xts[ch][:, a, :],
                                         start=(a == 0), stop=(a == 7))
                    kraw = workp.tile([128, TCH], F32, tag="kraw",
                                      name=f"kraw{ch}")
                    nc.vector.tensor_copy(kraw, psk)
                    sqk2 = workp.tile([128, TCH], F32R, tag="sqk2",
                                      name=f"sqk2_{ch}")
                    nc.gpsimd.tensor_mul(sqk2, kraw, kraw)
                    st["kraw"], st["sqk2"] = kraw, sqk2

                def stage_c():
                    psv = ps1p.tile([128, TCH], F32, tag="ps1",
                                    name=f"psv{ch}")
                    for a in range(8):
                        nc.tensor.matmul(psv, wv_sb[:, a, :],
                                         xts[ch][:, a, :],
                                         start=(a == 0), stop=(a == 7))
                    vt = workp.tile([128, TCH], BF16, tag="vt",
                                    name=f"vt{ch}")
                    nc.vector.tensor_copy(vt, psv)
                    for s5 in range(TCH // 128):
                        ptv = ps1p.tile([128, 128], BF16, tag="ps1",
                                        name=f"ptv{ch}_{s5}")
                        nc.tensor.transpose(ptv, vt[:, s5 * 128:
                                                    (s5 + 1) * 128], ident)
                        blk = (t0 + s5 * 128) // 128
                        nc.vector.tensor_copy(
                            v_sb[:, blk, :, 0:C],
                            ptv.rearrange("p (l c) -> p l c", l=LH))

                def stage_d():
                    # rstd: rows 0:2 = q heads, rows 2:4 = k heads, built by
                    # two accumulating matmuls with zero-masked selectors
                    ms = ps1p.tile([4, TCH], F32, tag="ps1",
                                   name=f"ms{ch}")
                    nc.tensor.matmul(ms, winv_sb[:, 0:4], st["sq2"],
                                     start=True, stop=False)
                    nc.tensor.matmul(ms, winv_sb[:, 4:8], st["sqk2"],
                                     start=False, stop=True)
                    lnv = workp.tile([4, TCH], F32, tag="lnv",
                                     name=f"lnv{ch}")
                    nc.scalar.activation(lnv, ms, AF.Ln,
                                         bias=eps128[0:4, :], scale=1.0 / C)
                    rstd = workp.tile([4, TCH], F32, tag="rstd",
                                      name=f"rstd{ch}")
                    nc.scalar.activation(rstd, lnv, AF.Exp, bias=0.0,
                                         scale=-0.5)
                    qdr = dram.tile([2, TCH], F32, tag="qdr", bufs=2,
                                    name=f"qdr{ch}")
                    kdr = dram.tile([2, TCH], F32, tag="kdr", bufs=2,
                                    name=f"kdr{ch}")
                    nc.sync.dma_start(out=qdr, in_=rstd[0:2, :])
                    nc.sync.dma_start(out=kdr, in_=rstd[2:4, :])
                    bqw = workp.tile([128, TCH], F32, tag="bqw",
                                     name=f"bqw{ch}")
                    nc.sync.dma_start(
                        out=bqw,
                        in_=bass.AP(tensor=qdr.tensor, offset=qdr.offset,
                                    ap=[[TCH, 2], [0, 64], [1, TCH]]))
                    bkw = workp.tile([128, TCH], F32, tag="bkw",
                                     name=f"bkw{ch}")
                    nc.sync.dma_start(
                        out=bkw,
                        in_=bass.AP(tensor=kdr.tensor, offset=kdr.offset,
                                    ap=[[TCH, 2], [0, 64], [1, TCH]]))

                    for which, raw, bw, dst in (
                            ("q", st["qraw"], bqw, qT_sb),
                            ("k", st["kraw"], bkw, k_sb)):
                        qn = workp.tile([128, TCH], F32, tag="qn",
                                        name=f"qn{which}{ch}")
                        nc.gpsimd.tensor_mul(qn, raw, bw)
                        t1 = workp.tile([128, TCH], F32, tag="t1",
                                        name=f"t1{which}{ch}")
                        nc.vector.tensor_mul(t1, ras[ch], qn)
                        rot = workp.tile([128, TCH], F32, tag="rot",
                                         name=f"rot{which}{ch}")
                        for g0 in (0, 64):
                            nc.gpsimd.dma_start(out=rot[g0:g0 + 32, :],
                                                in_=qn[g0 + 32:g0 + 64, :])
                            nc.gpsimd.dma_start(out=rot[g0 + 32:g0 + 64, :],
                                                in_=qn[g0:g0 + 32, :])
                        t2 = workp.tile([128, TCH], F32, tag="t2",
                                        name=f"t2{which}{ch}")
                        nc.gpsimd.tensor_mul(t2, rbs[ch], rot)
                        nc.gpsimd.tensor_add(dst[:, t0:t0 + TCH], t1, t2)

                return [stage_a, stage_b, stage_c, stage_d]

            # =============== phase-2 =====================================
            pos_tiles = {}

            def p2_start(c):
                pos_tiles[c] = [posp.tile([C + 1, TCH], F32, tag="pos",
                                          name=f"pos{c}_{lh}")
                                for lh in range(LH)]

            def p2_jt(c, jt):
                b = c // 4
                q0 = c * TCH
                j0 = b * S + jt * 128
                jblk = j0 // 128
                pss = pssp.tile([128, LH, TCH], F32, tag="pss",
                                name=f"pss{c}_{jt}")
                for lh in range(LH):
                    nc.tensor.matmul(
                        pss[:, lh, :],
                        k_sb[64 * lh:64 * lh + 64, j0:j0 + 128],
                        qT_sb[64 * lh:64 * lh + 64, q0:q0 + TCH],
                        start=True, stop=True)
                ex = workp.tile([128, LH, TCH], BF16, tag="ex", bufs=3,
                                name=f"ex{c}_{jt}")
                nc.scalar.activation(ex, pss, AF.Exp, bias=0.0, scale=0.125)
                for lh in range(LH):
                    nc.tensor.matmul(
                        pos_tiles[c][lh],
                        v_sb[:, jblk, lh, :],
                        ex[:, lh, :],
                        start=(jt == 0), stop=(jt == NJT - 1))

            def p2_norm(c):
                q = c // 2
                g0 = (c % 2) * 4
                for lh in range(LH):
                    po = pos_tiles[c][lh]
                    denr = workp.tile([1, TCH], F32, tag="denr",
                                      name=f"denr{c}_{lh}")
                    nc.vector.reciprocal(denr, po[C:C + 1, :])
                    dnb = workp.tile([C, TCH], F32, tag="dnb",
                                     name=f"dnb{c}_{lh}")
                    nc.gpsimd.partition_broadcast(dnb, denr, channels=C)
                    attbf = workp.tile([C, TCH], BF16, tag="attbf",
                                       name=f"attbf{c}_{lh}")
                    nc.vector.tensor_mul(attbf, po[0:C, :], dnb)
                    for d in range(4):
                        r0 = (g0 + d) * 128 + C * lh
                        nc.sync.dma_start(
                            out=bins[q][r0:r0 + C, :],
                            in_=attbf[:, d * 128:(d + 1) * 128])

            def collective(q):
                nc.gpsimd.collective_compute(
                    "AllToAll", mybir.AluOpType.bypass,
                    replica_groups=[list(range(NCORES))],
                    ins=[bins[q][:, :].opt()],
                    outs=[bouts[q][:, :].opt()])

            # =============== phase-3 (per quarter, 2 stages) ==============
            def p3_stages(q):
                st = {}

                def stage_a():
                    atta = workp.tile([128, 8, 128], BF16, tag="atta",
                                      name=f"atta{q}")
                    nc.sync.dma_start(
                        out=atta,
                        in_=bouts[q].rearrange("(g p) t -> p g t", p=128))
                    st["atta"] = atta

                def mk_nh(nh):
                    def stage_nh():
                        po3 = ps1p.tile([128, 512], F32, tag="ps1",
                                        name=f"po3_{q}_{nh}")
                        for a in range(8):
                            nc.tensor.matmul(
                                po3, st["atta"][:, a, :],
                                wo_sb[:, a, nh * 512:(nh + 1) * 512],
                                start=(a == 0), stop=(a == 7))
                        outsb = workp.tile([128, 512], F32, tag="outsb",
                                           name=f"outsb{q}_{nh}")
                        nc.vector.tensor_copy(outsb, po3)
                        nc.sync.dma_start(
                            out=out[q * 128:(q + 1) * 128,
                                    nh * 512:(nh + 1) * 512],
                            in_=outsb)
                    return stage_nh

                return [stage_a, mk_nh(0), mk_nh(1)]

            # =============== schedule ====================================
            load_xt(0)
            load_xt(1)
            load_rope(0)

            for ch in range(4):
                for stage in p1_stages(ch):
                    stage()

            # interleave queue: p1 chunks 4..7, then p3 quarters as they
            # become available
            ilq = []
            for ch in range(4, NCH):
                ilq.extend(p1_stages(ch))

            for c in range(NCH):
                p2_start(c)
                for jt in range(NJT):
                    p2_jt(c, jt)
                    if jt % 4 == 3 and ilq:
                        ilq.pop(0)()
                p2_norm(c)
                if c % 2 == 1:
                    q = c // 2
                    collective(q)
                    ilq.extend(p3_stages(q))
            while ilq:
                ilq.pop(0)()

    nc.compile()
    return nc


def kernel(x, rope_emb, Wq, Wk, Wv, q_norm_w, k_norm_w, Wout):
    global LAST_RESULTS
    if "nc" not in _CACHE:
        _CACHE["nc"] = _build()
    nc = _CACHE["nc"]

    # batch-major tokens: t = b*S + s
    x2 = np.ascontiguousarray(
        np.transpose(np.asarray(x, np.float32), (1, 0, 2)).reshape(T, D))
    xT_np = np.ascontiguousarray(x2.T)

    re = np.asarray(rope_emb, np.float32)
    cosT = np.ascontiguousarray(re[:, :, 0, 0].T)    # [32, S]
    r01T = np.ascontiguousarray(re[:, :, 0, 1].T)
    r10T = np.ascontiguousarray(re[:, :, 1, 0].T)
    cos2 = np.concatenate([cosT, cosT], axis=1)      # [32, T] batch-major
    r01_2 = np.concatenate([r01T, r01T], axis=1)
    r10_2 = np.concatenate([r10T, r10T], axis=1)
    ropeA_np = np.ascontiguousarray(
        np.concatenate([cos2, cos2, cos2, cos2], axis=0))
    ropeB_np = np.ascontiguousarray(
        np.concatenate([r01_2, r10_2, r01_2, r10_2], axis=0))

    qw_np = np.asarray(q_norm_w, np.float32)
    kw_np = np.asarray(k_norm_w, np.float32)
    # fold the RMSNorm weights into Wq/Wk columns (per dim_head channel)
    Wq_s = np.asarray(Wq, np.float32) * np.tile(qw_np, H)[None, :]
    Wk_s = np.asarray(Wk, np.float32) * np.tile(kw_np, H)[None, :]
    Wv = np.asarray(Wv, np.float32)
    Wout = np.ascontiguousarray(np.asarray(Wout, np.float32))

    # cols 0:4 = q-pass selector (k rows zero), cols 4:8 = k-pass selector
    winv_np = np.zeros((128, 8), np.float32)
    winv_np[0:64, 0] = 1.0 / (qw_np * qw_np)
    winv_np[64:128, 1] = 1.0 / (qw_np * qw_np)
    winv_np[0:64, 6] = 1.0 / (kw_np * kw_np)
    winv_np[64:128, 7] = 1.0 / (kw_np * kw_np)

    in_maps = []
    for g in range(NCORES):
        sl = slice(g * LC, (g + 1) * LC)
        in_maps.append({
            "xT": xT_np,
            "wq": np.ascontiguousarray(Wq_s[:, sl]),
            "wk": np.ascontiguousarray(Wk_s[:, sl]),
            "wv": np.ascontiguousarray(Wv[:, sl]),
            "wo": Wout,
            "ropeA": ropeA_np,
            "ropeB": ropeB_np,
            "winv": winv_np,
        })

    res = run_bass_kernel_spmd(nc, in_maps, core_ids=list(range(NCORES)))
    LAST_RESULTS = res
    # core g, quarter q holds tokens [(8q+g)*128, (8q+g+1)*128)
    out_full = np.empty((T, D), np.float32)
    for g in range(NCORES):
        og = res.results[g]["out"]
        for q in range(4):
            out_full[(8 * q + g) * 128:(8 * q + g + 1) * 128] = \
                og[q * 128:(q + 1) * 128]
    return np.ascontiguousarray(
        out_full.reshape(B, S, D).transpose(1, 0, 2))
